# revision 1
# baseline (speedup 1.0000x reference)
"""Trainium2 Bass kernel for nn_Conv_48679159332865 (Chebyshev spectral graph conv).

Algorithm (per core, data-parallel over the B*X*Y*Z dense dim):
  out = sum_k Cheb_k(L) @ x0 @ W_k + bias
evaluated via Clenshaw's backward recurrence so the Chebyshev basis never
needs transposing for the output projection:
  U_k = x0 @ W_k            (PE GEMMs contracting fin, from x0^T)
  B_k = U_k + 2 L B_{k+1} - B_{k+2}   for k = 6..1  (B_7 = U_7, B_8 = 0)
  S   = U_0 + L B_1 - B_2 + bias
The sparse laplacian is densified on the host (V=4096 -> 16M entries, 36864
nonzero) and streamed through the PE as bf16 tiles; all matmuls accumulate in
fp32 PSUM. Each of the 8 cores handles 27 of the 216 dense columns.
"""

import sys
from contextlib import ExitStack

import numpy as np
import ml_dtypes

for _p in ("/opt/trn_rl_repo", "/root/.axon_site/_ro/trn_rl_repo"):
    if _p not in sys.path:
        sys.path.insert(0, _p)

import concourse.bass as bass
import concourse.tile as tile
from concourse import mybir
from concourse.bass_utils import run_bass_kernel_spmd

FIN, V, FOUT, KK = 32, 4096, 32, 8
DP = 216            # B*X*Y*Z dense positions
NCORES = 8
DPC = DP // NCORES  # 27 dense positions per core
DC = DPC * FIN      # 864 working columns per core
NT = V // 128       # 32 v-tiles
BF16 = mybir.dt.bfloat16
F32 = mybir.dt.float32

_CACHE = {}


def _fix_excess_waits(nc, limit=1):
    """This walrus build supports one sync-wait per instruction; hoist excess
    waits onto NoOps inserted before the offending instruction."""
    for f in nc.m.functions:
        for blk in f.blocks:
            new_insts = []
            for inst in blk.instructions:
                si = inst.sync_info
                if si is not None and si.on_wait and len(si.on_wait) > limit:
                    waits = list(si.on_wait)
                    extra, keep = waits[:-limit], waits[-limit:]
                    for i in range(0, len(extra), limit):
                        nop = mybir.InstNoOp(
                            name=f"{inst.name}-waitsplit-{i}", ins=[], outs=[]
                        )
                        nop.engine = inst.engine
                        nop.sync_info = mybir.SyncInfo(
                            on_wait=extra[i : i + limit], on_update=[]
                        )
                        nc.register_instruction(nop, overwrite=True)
                        new_insts.append(nop)
                    inst.sync_info = mybir.SyncInfo(
                        on_wait=keep, on_update=list(si.on_update)
                    )
                new_insts.append(inst)
            blk.instructions[:] = new_insts


def _build_nc():
    nc = bass.Bass("TRN2", target_bir_lowering=False, debug=False)
    x0t = nc.dram_tensor("x0t", [DC, V], BF16, kind="ExternalInput")
    lt = nc.dram_tensor("lt", [NT, 128, NT, 128], BF16, kind="ExternalInput")
    wblk = nc.dram_tensor("wblk", [128, 1024], BF16, kind="ExternalInput")
    wblk6 = nc.dram_tensor("wblk6", [128, 768], BF16, kind="ExternalInput")
    brep = nc.dram_tensor("brep", [128, DC], F32, kind="ExternalInput")
    sout = nc.dram_tensor("sout", [V, DC], F32, kind="ExternalOutput")
    u = nc.dram_tensor("u", [KK, V, DC], BF16, kind="Internal")

    MUL = mybir.AluOpType.mult
    SUB = mybir.AluOpType.subtract

    with tile.TileContext(nc) as tc, ExitStack() as ctx:
        # ---------------- phase 1: U_k = x0 @ W_k for all k ----------------
        with (
            tc.tile_pool(name="uphase", bufs=1) as up,
            tc.tile_pool(name="ustg", bufs=2) as stgp,
            tc.tile_pool(name="upsum", bufs=2, space="PSUM") as ups,
        ):
            x0sb = up.tile([128, 7 * V], BF16)
            # rows 96:128 of the last (96-row) x0t tile are padding; zero them
            # so NaN garbage can't leak through the zero weight rows.
            nc.vector.memset(x0sb[96:128, 6 * V : 7 * V], 0.0)
            for r in range(7):
                rows = 128 if r < 6 else 96
                nc.sync.dma_start(
                    x0sb[0:rows, r * V : (r + 1) * V],
                    x0t.ap()[r * 128 : r * 128 + rows, :],
                )
            wsb = up.tile([128, 1024], BF16)
            nc.sync.dma_start(wsb[:], wblk.ap())
            w6sb = up.tile([128, 768], BF16)
            nc.sync.dma_start(w6sb[:], wblk6.ap())

            for vc in range(NT):
                stg = stgp.tile([128, KK * DC], BF16)
                for r in range(7):
                    ngrp = 4 if r < 6 else 3
                    N = 256 * ngrp
                    w = wsb if r < 6 else w6sb
                    ps = ups.tile([128, 1024], F32)
                    lhsT = x0sb[:, r * V + vc * 128 : r * V + vc * 128 + 128]
                    for off in range(0, N, 512):
                        n = min(512, N - off)
                        nc.tensor.matmul(
                            ps[:, off : off + n],
                            lhsT,
                            w[:, off : off + n],
                            start=True,
                            stop=True,
                        )
                    # psum cols are (g, k, fout); staging cols are (k, d', fout)
                    src = ps[:, 0:N].rearrange(
                        "p (g k f) -> p g k f", g=ngrp, k=KK, f=FOUT
                    )
                    dst = stg[:].rearrange("p (k d f) -> p d k f", k=KK, f=FOUT)[
                        :, 4 * r : 4 * r + ngrp
                    ]
                    nc.vector.tensor_copy(dst, src)
                nc.sync.dma_start(
                    u.ap()[:, vc * 128 : (vc + 1) * 128, :].rearrange(
                        "k v c -> v k c"
                    ),
                    stg[:].rearrange("p (k c) -> p k c", k=KK),
                )

        tc.strict_bb_all_engine_barrier()

        # ---------------- phase 2: Clenshaw backward sweep ----------------
        bp = ctx.enter_context(tc.tile_pool(name="bbuf", bufs=1))
        lsp = ctx.enter_context(tc.tile_pool(name="lstrip", bufs=3))
        usp = ctx.enter_context(tc.tile_pool(name="useg", bufs=3))
        tp = ctx.enter_context(tc.tile_pool(name="tmp", bufs=3))
        sp = ctx.enter_context(tc.tile_pool(name="spsum", bufs=3, space="PSUM"))
        outp = ctx.enter_context(tc.tile_pool(name="souttile", bufs=2))

        bufA = bp.tile([128, NT * DC], BF16, tag="bufA")
        bufB = bp.tile([128, NT * DC], BF16, tag="bufB")
        bsb = bp.tile([128, DC], F32, tag="bsb")
        nc.sync.dma_start(bsb[:], brep.ap())

        # B_7 = U_7
        for I in range(NT):
            nc.sync.dma_start(
                bufB[:, I * DC : (I + 1) * DC], u.ap()[7, I * 128 : (I + 1) * 128, :]
            )

        def spmm_tile(I, rhs):
            """PSUM tile <- (L @ B)[I*128:(I+1)*128, :]"""
            lstrip = lsp.tile([128, NT * 128], BF16)
            nc.sync.dma_start(
                lstrip[:], lt.ap()[I].rearrange("p kt j -> p (kt j)")
            )
            ps = sp.tile([128, DC], F32)
            for kt in range(NT):
                lh = lstrip[:, kt * 128 : (kt + 1) * 128]
                for off in (0, 512):
                    n = min(512, DC - off)
                    nc.tensor.matmul(
                        ps[:, off : off + n],
                        lh,
                        rhs[:, kt * DC + off : kt * DC + off + n],
                        start=(kt == 0),
                        stop=(kt == NT - 1),
                    )
            return ps

        for s, k in enumerate(range(6, 0, -1)):
            rhs = bufB if s % 2 == 0 else bufA
            wr = bufA if s % 2 == 0 else bufB
            for I in range(NT):
                ps = spmm_tile(I, rhs)
                useg = usp.tile([128, DC], BF16)
                nc.sync.dma_start(
                    useg[:], u.ap()[k, I * 128 : (I + 1) * 128, :]
                )
                t = tp.tile([128, DC], F32)
                if s == 0:  # B_{k+2} = 0
                    nc.scalar.mul(t[:], ps[:], 2.0)
                else:
                    nc.vector.scalar_tensor_tensor(
                        t[:], ps[:], 2.0, wr[:, I * DC : (I + 1) * DC], MUL, SUB
                    )
                nc.vector.tensor_add(wr[:, I * DC : (I + 1) * DC], t[:], useg[:])

        # S = U_0 + L B_1 - B_2 + bias;  B_1 = bufB, B_2 = bufA
        for I in range(NT):
            ps = spmm_tile(I, bufB)
            useg = usp.tile([128, DC], BF16)
            nc.sync.dma_start(useg[:], u.ap()[0, I * 128 : (I + 1) * 128, :])
            t = tp.tile([128, DC], F32)
            nc.vector.scalar_tensor_tensor(
                t[:], ps[:], 1.0, bufA[:, I * DC : (I + 1) * DC], MUL, SUB
            )
            t2 = tp.tile([128, DC], F32, tag="t2")
            nc.vector.tensor_add(t2[:], t[:], useg[:])
            st = outp.tile([128, DC], F32)
            nc.vector.tensor_add(st[:], t2[:], bsb[:])
            nc.sync.dma_start(sout.ap()[I * 128 : (I + 1) * 128, :], st[:])

    _fix_excess_waits(nc)
    return nc


def _host_prep(inputs, weight, bias, lap_vals, lap_rows, lap_cols):
    bf = ml_dtypes.bfloat16
    # dense L, blocked+transposed for PE stationary tiles:
    # lt[I, p, kt, j] = L[128I+j, kt*128+p]
    L = np.zeros((V, V), dtype=np.float32)
    np.add.at(L, (lap_rows, lap_cols), lap_vals.astype(np.float32))
    lt = np.ascontiguousarray(
        L.reshape(NT, 128, NT, 128).transpose(0, 3, 2, 1).astype(bf)
    )

    # block-diagonal weights for the U GEMMs: 4 (or 3) d'-groups of 32 fin
    # rows each, mapped to 256 (k,fout) output columns per group.
    Wf = weight.astype(np.float32).transpose(1, 0, 2).reshape(FIN, KK * FOUT)
    wblk = np.zeros((128, 1024), dtype=np.float32)
    wblk6 = np.zeros((128, 768), dtype=np.float32)
    for g in range(4):
        wblk[g * 32 : (g + 1) * 32, g * 256 : (g + 1) * 256] = Wf
        if g < 3:
            wblk6[g * 32 : (g + 1) * 32, g * 256 : (g + 1) * 256] = Wf
    wblk = wblk.astype(bf)
    wblk6 = wblk6.astype(bf)

    brep = np.broadcast_to(
        np.tile(bias.astype(np.float32), DPC)[None, :], (128, DC)
    ).copy()

    # per-core x0^T shards: x0t_m[d'*32+fin, v] = inputs[0, fin, v, 27m+d']
    x = np.asarray(inputs, dtype=np.float32).reshape(FIN, V, DP)
    in_maps = []
    for m in range(NCORES):
        xm = x[:, :, DPC * m : DPC * (m + 1)]  # [fin, v, d']
        x0t_m = np.ascontiguousarray(
            xm.transpose(2, 0, 1).reshape(DC, V).astype(bf)
        )
        in_maps.append(
            {"x0t": x0t_m, "lt": lt, "wblk": wblk, "wblk6": wblk6, "brep": brep}
        )
    return in_maps


def kernel(inputs, weight, bias, lap_vals, lap_rows, lap_cols):
    if "nc" not in _CACHE:
        _CACHE["nc"] = _build_nc()
    nc = _CACHE["nc"]
    in_maps = _host_prep(inputs, weight, bias, lap_vals, lap_rows, lap_cols)
    res = run_bass_kernel_spmd(nc, in_maps, core_ids=list(range(NCORES)))
    out = np.zeros((FOUT, V, DP), dtype=np.float32)
    for m in range(NCORES):
        S = res.results[m]["sout"].reshape(V, DPC, FOUT)
        out[:, :, DPC * m : DPC * (m + 1)] = S.transpose(2, 0, 1)
    return out.reshape(1, FOUT, V, 6, 6, 6)
